# revision 30
# baseline (speedup 1.0000x reference)
"""Trainium2 Bass kernel for CombineRadialSpeciesWithAngularAdaptBasis.

Computation: for l in 0..5 (m = 2l+1):
    o_l = einsum('smp,pb->smb', values_l [N,m,P], W_l [P,B])   -> reshape (N*m, B)
    g_l = einsum('sxmp,pb->sxmb', grads_l [NG,3,m,P], W_l)     -> reshape (NG*3*m, B)
  output = concat([o_0, g_0, o_1, g_1, ... o_5, g_5], axis=0)

Strategy: data-parallel across samples on 8 NeuronCores. The kernel is pure
streaming GEMM with tiny stationary weights, so it is HBM-DMA bound; all
device I/O is bf16 (halves traffic vs f32; rel-err ~1e-3, tolerance 2e-2).
Host packs each core's shard into ONE contiguous x^T [P=80, S=243000] bf16
tensor (regions back-to-back in output order); on-chip, W_l [80,64] bf16 is
the stationary operand, x^T streams through the PE in 512-col tiles (PSUM
f32), PSUM is downcast-copied to bf16 SBUF and DMA'd to a flat y [64, S]
bf16, which the host transposes/upcasts back.
"""
import numpy as np
import ml_dtypes

BF16 = np.dtype(ml_dtypes.bfloat16)

N, NG, P, B, LMAX = 30000, 8000, 80, 64, 5
NCORES = 8
NV = N // NCORES      # 3750 values samples per core
NGV = NG // NCORES    # 1000 grads samples per core

CHUNK = 8192          # columns per DMA chunk (80 x 16KB lines in)
NT = 512              # matmul moving-operand tile (one PSUM bank fp32)

# Region order matches the reference's output concatenation: v0,g0,v1,g1,...
# Each entry: (is_grad, l, columns per core)
REGIONS = []
for _l in range(LMAX + 1):
    _m = 2 * _l + 1
    REGIONS.append((False, _l, NV * _m))
    REGIONS.append((True, _l, NGV * 3 * _m))
STOT = sum(r[2] for r in REGIONS)  # 243000

# Flat-column segments: (start_col, end_col, l)
SEGS = []
_off = 0
for _g, _l, _cols in REGIONS:
    SEGS.append((_off, _off + _cols, _l))
    _off += _cols

# chunk schedule: a small first chunk fills the pipeline sooner, then
# full-size chunks, then the natural ragged tail
CHUNKS = []
_c = 0
for _sz in [CHUNK] * STOT:
    if _c >= STOT:
        break
    _sz = min(_sz, STOT - _c)
    CHUNKS.append((_c, _sz))
    _c += _sz

_CACHE = {}


def _segments_in(lo, hi):
    """Yield (s, e, l) sub-intervals of [lo, hi) split at region bounds."""
    for s, e, l in SEGS:
        a, b = max(s, lo), min(e, hi)
        if a < b:
            yield a, b, l


def _build_program():
    """Build and finalize the (SPMD, per-core) Bass program once."""
    import concourse.bass as bass
    import concourse.tile as tile
    import concourse.mybir as mybir
    from concourse import bacc

    f32 = mybir.dt.float32
    bf16 = mybir.dt.bfloat16

    nc = bacc.Bacc("TRN2", target_bir_lowering=False, debug=False,
                   num_devices=NCORES)
    x = nc.declare_dram_parameter("x", [P, STOT], bf16, isOutput=False)
    # all six W_l packed side by side -> one DMA, off the input (sync) ring
    wall = nc.declare_dram_parameter("wall", [P, (LMAX + 1) * B], bf16,
                                     isOutput=False)
    y = nc.declare_dram_parameter("y", [B, STOT], bf16, isOutput=True)

    with tile.TileContext(nc) as tc:
        with (
            tc.tile_pool(name="wp", bufs=1) as wp,
            tc.tile_pool(name="inp", bufs=6) as inp,
            tc.tile_pool(name="outp", bufs=4) as outp,
            tc.tile_pool(name="psp", bufs=8, space="PSUM") as psp,
        ):
            wt = wp.tile([P, (LMAX + 1) * B], bf16, name="wt", tag="wt")
            nc.scalar.dma_start(wt[:], wall[:, :])
            w_sb = [wt[:, l * B:(l + 1) * B] for l in range(LMAX + 1)]

            for ci, (c0, csz) in enumerate(CHUNKS):
                xt = inp.tile([P, csz], bf16, name=f"xt_{ci}", tag="xt")
                # reads alternate across both HWDGE rings for deeper
                # per-engine read pipelining; writes go via SWDGE below
                if ci % 2 == 0:
                    nc.sync.dma_start(xt[:], x[:, c0:c0 + csz])
                else:
                    nc.scalar.dma_start(xt[:], x[:, c0:c0 + csz])
                ot = outp.tile([B, csz], bf16, name=f"ot_{ci}", tag="ot")
                for ti, k0 in enumerate(range(0, csz, NT)):
                    n = min(NT, csz - k0)
                    ps = psp.tile([B, n], f32, name=f"ps_{ci}_{k0}", tag="ps")
                    for sa, sb, l in _segments_in(c0 + k0, c0 + k0 + n):
                        ra, rb = sa - c0, sb - c0
                        nc.tensor.matmul(ps[:, ra - k0:rb - k0],
                                         lhsT=w_sb[l],
                                         rhs=xt[:, ra:rb],
                                         start=True, stop=True)
                    # all copies of one chunk on one engine so the output
                    # DMA needs a single sync wait; alternate per chunk
                    if ci % 2 == 0:
                        nc.vector.tensor_copy(ot[:, k0:k0 + n], ps[:])
                    else:
                        nc.scalar.copy(ot[:, k0:k0 + n], ps[:])
                nc.gpsimd.dma_start(y[:, c0:c0 + csz], ot[:])

    nc.finalize()
    return nc


def _get_program():
    if "nc" not in _CACHE:
        _CACHE["nc"] = _build_program()
    return _CACHE["nc"]


def _register_ntff_hook():
    """antenv.axon_hooks is absent in this image; the .so supports NTFF
    profiling — install the shim so run_bass_kernel_spmd(trace=True) works."""
    import sys, types
    try:
        from antenv.axon_hooks import get_axon_ntff_profile_hook  # noqa: F401
        return
    except ImportError:
        pass
    import antenv
    from trn_agent_boot.trn_boot import _ntff_profile_via_ctypes
    mod = types.ModuleType("antenv.axon_hooks")
    mod._hook = _ntff_profile_via_ctypes('/opt/axon/libaxon_pjrt.so')
    mod.get_axon_ntff_profile_hook = lambda: mod._hook
    mod.set_axon_ntff_profile_hook = lambda h: setattr(mod, '_hook', h)
    sys.modules["antenv.axon_hooks"] = mod
    antenv.axon_hooks = mod


LAST_EXEC_TIME_NS = None
LAST_MEAN_EXEC_TIME_NS = None


def kernel(trace=False, trace_all_cores=False, **inputs):
    global LAST_EXEC_TIME_NS, LAST_MEAN_EXEC_TIME_NS
    from concourse.bass_utils import run_bass_kernel_spmd

    # ---- host-side: shard, transpose to [P, S], pack flat, cast bf16 ----
    wall = np.concatenate(
        [np.asarray(inputs[f"W_l{l}"]) for l in range(LMAX + 1)],
        axis=1).astype(BF16)
    in_maps = [{"x": np.empty((P, STOT), dtype=BF16), "wall": wall}
               for _ in range(NCORES)]
    off = 0
    for g, l, cols in REGIONS:
        src = inputs[f"grads_l{l}"] if g else inputs[f"values_l{l}"]
        src = np.asarray(src)
        ns = NGV if g else NV
        for i in range(NCORES):
            blk = src[i * ns:(i + 1) * ns].reshape(cols, P).astype(BF16)
            in_maps[i]["x"][:, off:off + cols] = blk.T
        off += cols

    nc = _get_program()
    kwargs = {}
    if trace:
        _register_ntff_hook()
        kwargs["trace"] = True
        if trace_all_cores:
            kwargs["trace_cores"] = list(range(NCORES))
    res = run_bass_kernel_spmd(nc, in_maps, list(range(NCORES)), **kwargs)
    LAST_EXEC_TIME_NS = res.exec_time_ns
    LAST_MEAN_EXEC_TIME_NS = res.mean_exec_time_ns

    # ---- gather: transpose each region back, upcast, concatenate ----
    outs = [res.results[i]["y"] for i in range(NCORES)]
    total_rows = NCORES * STOT
    final = np.empty((total_rows, B), dtype=np.float32)
    row = 0
    off = 0
    for g, l, cols in REGIONS:
        for i in range(NCORES):
            final[row:row + cols] = outs[i][:, off:off + cols].T
            row += cols
        off += cols
    return final


# revision 31
# speedup vs baseline: 1.0700x; 1.0700x over previous
"""Trainium2 Bass kernel for CombineRadialSpeciesWithAngularAdaptBasis.

Computation: for l in 0..5 (m = 2l+1):
    o_l = einsum('smp,pb->smb', values_l [N,m,P], W_l [P,B])   -> reshape (N*m, B)
    g_l = einsum('sxmp,pb->sxmb', grads_l [NG,3,m,P], W_l)     -> reshape (NG*3*m, B)
  output = concat([o_0, g_0, o_1, g_1, ... o_5, g_5], axis=0)

Strategy: data-parallel across samples on 8 NeuronCores. The kernel is pure
streaming GEMM with tiny stationary weights, so it is HBM-DMA bound; all
device I/O is bf16 (halves traffic vs f32; rel-err ~1e-3, tolerance 2e-2).
Host packs each core's shard into ONE contiguous x^T [P=80, S=243000] bf16
tensor (regions back-to-back in output order); on-chip, W_l [80,64] bf16 is
the stationary operand, x^T streams through the PE in 512-col tiles (PSUM
f32), PSUM is downcast-copied to bf16 SBUF and DMA'd to a flat y [64, S]
bf16, which the host transposes/upcasts back.
"""
import numpy as np
import ml_dtypes

BF16 = np.dtype(ml_dtypes.bfloat16)

N, NG, P, B, LMAX = 30000, 8000, 80, 64, 5
NCORES = 8
NV = N // NCORES      # 3750 values samples per core
NGV = NG // NCORES    # 1000 grads samples per core

CHUNK = 8192          # columns per DMA chunk (80 x 16KB lines in)
NT = 512              # matmul moving-operand tile (one PSUM bank fp32)

# Region order matches the reference's output concatenation: v0,g0,v1,g1,...
# Each entry: (is_grad, l, columns per core)
REGIONS = []
for _l in range(LMAX + 1):
    _m = 2 * _l + 1
    REGIONS.append((False, _l, NV * _m))
    REGIONS.append((True, _l, NGV * 3 * _m))
STOT = sum(r[2] for r in REGIONS)  # 243000

# Flat-column segments: (start_col, end_col, l)
SEGS = []
_off = 0
for _g, _l, _cols in REGIONS:
    SEGS.append((_off, _off + _cols, _l))
    _off += _cols

# chunk schedule: a small first chunk fills the pipeline sooner, then
# full-size chunks, then the natural ragged tail
CHUNKS = []
_c = 0
for _sz in [CHUNK] * STOT:
    if _c >= STOT:
        break
    _sz = min(_sz, STOT - _c)
    CHUNKS.append((_c, _sz))
    _c += _sz

_CACHE = {}


def _segments_in(lo, hi):
    """Yield (s, e, l) sub-intervals of [lo, hi) split at region bounds."""
    for s, e, l in SEGS:
        a, b = max(s, lo), min(e, hi)
        if a < b:
            yield a, b, l


def _build_program():
    """Build and finalize the (SPMD, per-core) Bass program once."""
    import concourse.bass as bass
    import concourse.tile as tile
    import concourse.mybir as mybir
    from concourse import bacc

    f32 = mybir.dt.float32
    bf16 = mybir.dt.bfloat16

    nc = bacc.Bacc("TRN2", target_bir_lowering=False, debug=False,
                   num_devices=NCORES)
    x = nc.declare_dram_parameter("x", [P, STOT], bf16, isOutput=False)
    # all six W_l packed side by side -> one DMA, off the input (sync) ring
    wall = nc.declare_dram_parameter("wall", [P, (LMAX + 1) * B], bf16,
                                     isOutput=False)
    y = nc.declare_dram_parameter("y", [B, STOT], bf16, isOutput=True)

    with tile.TileContext(nc) as tc:
        with (
            tc.tile_pool(name="wp", bufs=1) as wp,
            tc.tile_pool(name="inp", bufs=4) as inp,
            tc.tile_pool(name="outp", bufs=4) as outp,
            tc.tile_pool(name="psp", bufs=8, space="PSUM") as psp,
        ):
            wt = wp.tile([P, (LMAX + 1) * B], bf16, name="wt", tag="wt")
            nc.scalar.dma_start(wt[:], wall[:, :])
            w_sb = [wt[:, l * B:(l + 1) * B] for l in range(LMAX + 1)]

            for ci, (c0, csz) in enumerate(CHUNKS):
                xt = inp.tile([P, csz], bf16, name=f"xt_{ci}", tag="xt")
                # reads alternate across both HWDGE rings for deeper
                # per-engine read pipelining; writes go via SWDGE below
                if ci % 2 == 0:
                    nc.sync.dma_start(xt[:], x[:, c0:c0 + csz])
                else:
                    nc.scalar.dma_start(xt[:], x[:, c0:c0 + csz])
                ot = outp.tile([B, csz], bf16, name=f"ot_{ci}", tag="ot")
                for ti, k0 in enumerate(range(0, csz, NT)):
                    n = min(NT, csz - k0)
                    ps = psp.tile([B, n], f32, name=f"ps_{ci}_{k0}", tag="ps")
                    for sa, sb, l in _segments_in(c0 + k0, c0 + k0 + n):
                        ra, rb = sa - c0, sb - c0
                        nc.tensor.matmul(ps[:, ra - k0:rb - k0],
                                         lhsT=w_sb[l],
                                         rhs=xt[:, ra:rb],
                                         start=True, stop=True)
                    # all copies of one chunk on one engine so the output
                    # DMA needs a single sync wait; alternate per chunk
                    if ci % 2 == 0:
                        nc.vector.tensor_copy(ot[:, k0:k0 + n], ps[:])
                    else:
                        nc.scalar.copy(ot[:, k0:k0 + n], ps[:])
                nc.gpsimd.dma_start(y[:, c0:c0 + csz], ot[:])

    nc.finalize()
    return nc


def _get_program():
    if "nc" not in _CACHE:
        _CACHE["nc"] = _build_program()
    return _CACHE["nc"]


def _register_ntff_hook():
    """antenv.axon_hooks is absent in this image; the .so supports NTFF
    profiling — install the shim so run_bass_kernel_spmd(trace=True) works."""
    import sys, types
    try:
        from antenv.axon_hooks import get_axon_ntff_profile_hook  # noqa: F401
        return
    except ImportError:
        pass
    import antenv
    from trn_agent_boot.trn_boot import _ntff_profile_via_ctypes
    mod = types.ModuleType("antenv.axon_hooks")
    mod._hook = _ntff_profile_via_ctypes('/opt/axon/libaxon_pjrt.so')
    mod.get_axon_ntff_profile_hook = lambda: mod._hook
    mod.set_axon_ntff_profile_hook = lambda h: setattr(mod, '_hook', h)
    sys.modules["antenv.axon_hooks"] = mod
    antenv.axon_hooks = mod


LAST_EXEC_TIME_NS = None
LAST_MEAN_EXEC_TIME_NS = None


def kernel(trace=False, trace_all_cores=False, **inputs):
    global LAST_EXEC_TIME_NS, LAST_MEAN_EXEC_TIME_NS
    from concourse.bass_utils import run_bass_kernel_spmd

    # ---- host-side: shard, transpose to [P, S], pack flat, cast bf16 ----
    wall = np.concatenate(
        [np.asarray(inputs[f"W_l{l}"]) for l in range(LMAX + 1)],
        axis=1).astype(BF16)
    in_maps = [{"x": np.empty((P, STOT), dtype=BF16), "wall": wall}
               for _ in range(NCORES)]
    off = 0
    for g, l, cols in REGIONS:
        src = inputs[f"grads_l{l}"] if g else inputs[f"values_l{l}"]
        src = np.asarray(src)
        ns = NGV if g else NV
        for i in range(NCORES):
            blk = src[i * ns:(i + 1) * ns].reshape(cols, P).astype(BF16)
            in_maps[i]["x"][:, off:off + cols] = blk.T
        off += cols

    nc = _get_program()
    kwargs = {}
    if trace:
        _register_ntff_hook()
        kwargs["trace"] = True
        if trace_all_cores:
            kwargs["trace_cores"] = list(range(NCORES))
    res = run_bass_kernel_spmd(nc, in_maps, list(range(NCORES)), **kwargs)
    LAST_EXEC_TIME_NS = res.exec_time_ns
    LAST_MEAN_EXEC_TIME_NS = res.mean_exec_time_ns

    # ---- gather: transpose each region back, upcast, concatenate ----
    outs = [res.results[i]["y"] for i in range(NCORES)]
    total_rows = NCORES * STOT
    final = np.empty((total_rows, B), dtype=np.float32)
    row = 0
    off = 0
    for g, l, cols in REGIONS:
        for i in range(NCORES):
            final[row:row + cols] = outs[i][:, off:off + cols].T
            row += cols
        off += cols
    return final


# revision 32
# speedup vs baseline: 1.0812x; 1.0104x over previous
"""Trainium2 Bass kernel for CombineRadialSpeciesWithAngularAdaptBasis.

Computation: for l in 0..5 (m = 2l+1):
    o_l = einsum('smp,pb->smb', values_l [N,m,P], W_l [P,B])   -> reshape (N*m, B)
    g_l = einsum('sxmp,pb->sxmb', grads_l [NG,3,m,P], W_l)     -> reshape (NG*3*m, B)
  output = concat([o_0, g_0, o_1, g_1, ... o_5, g_5], axis=0)

Strategy: data-parallel across samples on 8 NeuronCores. The kernel is pure
streaming GEMM with tiny stationary weights, so it is HBM-DMA bound; all
device I/O is bf16 (halves traffic vs f32; rel-err ~1e-3, tolerance 2e-2).
Host packs each core's shard into ONE contiguous x^T [P=80, S=243000] bf16
tensor (regions back-to-back in output order); on-chip, W_l [80,64] bf16 is
the stationary operand, x^T streams through the PE in 512-col tiles (PSUM
f32), PSUM is downcast-copied to bf16 SBUF and DMA'd to a flat y [64, S]
bf16, which the host transposes/upcasts back.
"""
import numpy as np
import ml_dtypes

BF16 = np.dtype(ml_dtypes.bfloat16)

N, NG, P, B, LMAX = 30000, 8000, 80, 64, 5
NCORES = 8
NV = N // NCORES      # 3750 values samples per core
NGV = NG // NCORES    # 1000 grads samples per core

CHUNK = 8192          # columns per DMA chunk (80 x 16KB lines in)
NT = 512              # matmul moving-operand tile (one PSUM bank fp32)

# Region order matches the reference's output concatenation: v0,g0,v1,g1,...
# Each entry: (is_grad, l, columns per core)
REGIONS = []
for _l in range(LMAX + 1):
    _m = 2 * _l + 1
    REGIONS.append((False, _l, NV * _m))
    REGIONS.append((True, _l, NGV * 3 * _m))
STOT = sum(r[2] for r in REGIONS)  # 243000

# Flat-column segments: (start_col, end_col, l)
SEGS = []
_off = 0
for _g, _l, _cols in REGIONS:
    SEGS.append((_off, _off + _cols, _l))
    _off += _cols

# chunk schedule: small chunks at the head fill the pipeline sooner; the
# tail shrinks so the final copies + output write drain quickly
_SIZES = ([2048, 4096] + [CHUNK] * 27
          + [4096, 4096, 2048, 2048, 2048, 1336])
assert sum(_SIZES) == STOT
CHUNKS = []
_c = 0
for _sz in _SIZES:
    CHUNKS.append((_c, _sz))
    _c += _sz

_CACHE = {}


def _segments_in(lo, hi):
    """Yield (s, e, l) sub-intervals of [lo, hi) split at region bounds."""
    for s, e, l in SEGS:
        a, b = max(s, lo), min(e, hi)
        if a < b:
            yield a, b, l


def _build_program():
    """Build and finalize the (SPMD, per-core) Bass program once."""
    import concourse.bass as bass
    import concourse.tile as tile
    import concourse.mybir as mybir
    from concourse import bacc

    f32 = mybir.dt.float32
    bf16 = mybir.dt.bfloat16

    nc = bacc.Bacc("TRN2", target_bir_lowering=False, debug=False,
                   num_devices=NCORES)
    x = nc.declare_dram_parameter("x", [P, STOT], bf16, isOutput=False)
    # all six W_l packed side by side -> one DMA, off the input (sync) ring
    wall = nc.declare_dram_parameter("wall", [P, (LMAX + 1) * B], bf16,
                                     isOutput=False)
    y = nc.declare_dram_parameter("y", [B, STOT], bf16, isOutput=True)

    with tile.TileContext(nc) as tc:
        with (
            tc.tile_pool(name="wp", bufs=1) as wp,
            tc.tile_pool(name="inp", bufs=4) as inp,
            tc.tile_pool(name="outp", bufs=4) as outp,
            tc.tile_pool(name="psp", bufs=8, space="PSUM") as psp,
        ):
            wt = wp.tile([P, (LMAX + 1) * B], bf16, name="wt", tag="wt")
            nc.scalar.dma_start(wt[:], wall[:, :])
            w_sb = [wt[:, l * B:(l + 1) * B] for l in range(LMAX + 1)]

            for ci, (c0, csz) in enumerate(CHUNKS):
                xt = inp.tile([P, csz], bf16, name=f"xt_{ci}", tag="xt")
                # reads alternate across both HWDGE rings for deeper
                # per-engine read pipelining; writes go via SWDGE below
                if ci % 2 == 0:
                    nc.sync.dma_start(xt[:], x[:, c0:c0 + csz])
                else:
                    nc.scalar.dma_start(xt[:], x[:, c0:c0 + csz])
                ot = outp.tile([B, csz], bf16, name=f"ot_{ci}", tag="ot")
                for ti, k0 in enumerate(range(0, csz, NT)):
                    n = min(NT, csz - k0)
                    ps = psp.tile([B, n], f32, name=f"ps_{ci}_{k0}", tag="ps")
                    for sa, sb, l in _segments_in(c0 + k0, c0 + k0 + n):
                        ra, rb = sa - c0, sb - c0
                        nc.tensor.matmul(ps[:, ra - k0:rb - k0],
                                         lhsT=w_sb[l],
                                         rhs=xt[:, ra:rb],
                                         start=True, stop=True)
                    # all copies of one chunk on one engine so the output
                    # DMA needs a single sync wait; alternate per chunk
                    if ci % 2 == 0:
                        nc.vector.tensor_copy(ot[:, k0:k0 + n], ps[:])
                    else:
                        nc.scalar.copy(ot[:, k0:k0 + n], ps[:])
                nc.gpsimd.dma_start(y[:, c0:c0 + csz], ot[:])

    nc.finalize()
    return nc


def _get_program():
    if "nc" not in _CACHE:
        _CACHE["nc"] = _build_program()
    return _CACHE["nc"]


def _register_ntff_hook():
    """antenv.axon_hooks is absent in this image; the .so supports NTFF
    profiling — install the shim so run_bass_kernel_spmd(trace=True) works."""
    import sys, types
    try:
        from antenv.axon_hooks import get_axon_ntff_profile_hook  # noqa: F401
        return
    except ImportError:
        pass
    import antenv
    from trn_agent_boot.trn_boot import _ntff_profile_via_ctypes
    mod = types.ModuleType("antenv.axon_hooks")
    mod._hook = _ntff_profile_via_ctypes('/opt/axon/libaxon_pjrt.so')
    mod.get_axon_ntff_profile_hook = lambda: mod._hook
    mod.set_axon_ntff_profile_hook = lambda h: setattr(mod, '_hook', h)
    sys.modules["antenv.axon_hooks"] = mod
    antenv.axon_hooks = mod


LAST_EXEC_TIME_NS = None
LAST_MEAN_EXEC_TIME_NS = None


def kernel(trace=False, trace_all_cores=False, **inputs):
    global LAST_EXEC_TIME_NS, LAST_MEAN_EXEC_TIME_NS
    from concourse.bass_utils import run_bass_kernel_spmd

    # ---- host-side: shard, transpose to [P, S], pack flat, cast bf16 ----
    wall = np.concatenate(
        [np.asarray(inputs[f"W_l{l}"]) for l in range(LMAX + 1)],
        axis=1).astype(BF16)
    in_maps = [{"x": np.empty((P, STOT), dtype=BF16), "wall": wall}
               for _ in range(NCORES)]
    off = 0
    for g, l, cols in REGIONS:
        src = inputs[f"grads_l{l}"] if g else inputs[f"values_l{l}"]
        src = np.asarray(src)
        ns = NGV if g else NV
        for i in range(NCORES):
            blk = src[i * ns:(i + 1) * ns].reshape(cols, P).astype(BF16)
            in_maps[i]["x"][:, off:off + cols] = blk.T
        off += cols

    nc = _get_program()
    kwargs = {}
    if trace:
        _register_ntff_hook()
        kwargs["trace"] = True
        if trace_all_cores:
            kwargs["trace_cores"] = list(range(NCORES))
    res = run_bass_kernel_spmd(nc, in_maps, list(range(NCORES)), **kwargs)
    LAST_EXEC_TIME_NS = res.exec_time_ns
    LAST_MEAN_EXEC_TIME_NS = res.mean_exec_time_ns

    # ---- gather: transpose each region back, upcast, concatenate ----
    outs = [res.results[i]["y"] for i in range(NCORES)]
    total_rows = NCORES * STOT
    final = np.empty((total_rows, B), dtype=np.float32)
    row = 0
    off = 0
    for g, l, cols in REGIONS:
        for i in range(NCORES):
            final[row:row + cols] = outs[i][:, off:off + cols].T
            row += cols
        off += cols
    return final
